# revision 1
# baseline (speedup 1.0000x reference)
"""CantorGlobalAttention Trainium2 kernel, v2.

Dense-masked routed attention (multiplicity mask M reproduces the softmax
over 64 route slots exactly), restructured vs v1:

- Hybrid sharding: 8 cores = 2 batches x 2 head-halves x 2 seq-halves.
  Each core: QKV projection for its 4 heads only, dense masked attention
  for its 1024 queries, and a PARTIAL output projection (contraction over
  its 4 heads' 256 ao-dims). Host sums the two head-half partials + b_proj.
- Transposed AV: attn weights (atm, [j, q] layout) are the matmul
  STATIONARY operand, V the moving operand, so the AV output lands as
  [q, head_dim+1] using all 128 partitions (half the PE rows of v1), and
  the softmax denominator (ones column) becomes a per-partition scalar:
  normalization is a DVE reciprocal + per-partition-scale multiply
  (no ln/exp on Act, no DRAM-bounce broadcast).
- Biases for K/Q folded into the PSUM->SBUF copies (per-partition
  tensor_scalar add on the gpsimd/Pool engine).
- bf16 storage for x, weights, mask, K/Q/V, attn weights, ao.
- ao is PE-transposed (identity matmul) to feed the output projection.
"""

import sys

try:
    import concourse.bass as bass  # noqa: F401
except Exception:  # pragma: no cover
    sys.path.insert(0, "/opt/trn_rl_repo")

import numpy as np
import ml_dtypes

import concourse.bass as bass
import concourse.mybir as mybir
import concourse.tile as tile
from concourse.bass_utils import run_bass_kernel_spmd
from concourse.vector_clock import ScopedClock

dt = mybir.dt
AF = mybir.ActivationFunctionType
ALU = mybir.AluOpType

S = 2048
D = 512
H = 8
HD = 64
B = 2
NCORES = 8
QS = 1024            # queries per core
HH = 4               # heads per core
SCALE = HD ** -0.5   # 0.125
NJT = S // 128       # 16 j-tiles
NCHUNK = NJT // 2    # 8 chunks of 2 j-tiles


# ---------------------------------------------------------------------------
# walrus workaround: this walrus build accepts at most ONE sync-wait command
# per instruction; hoist extras onto same-engine nop carriers.
# ---------------------------------------------------------------------------
def _patched_drain_and_barrier(self, tick_clock, wait_clock):
    nc = self.nc
    drain_inst = nc.sync.drain()
    wait_clock.add_sem_waits(
        drain_inst.ins, ScopedClock({None: tick_clock.global_clock})
    )
    nc.all_engine_barrier()
    assert self.sems is not None
    popped = nc._tile_sem_poison_stack.pop()
    assert popped is self._sem_poison
    nc.clear_and_free_semaphores(list(self.sems.allocated().values()))
    nc.all_engine_barrier()


tile.TileContext._drain_and_barrier = _patched_drain_and_barrier


def _split_sync_waits(nc, maxw=1):
    n_fixed = 0
    for fn in nc.m.functions:
        for bb in fn.blocks:
            src = list(bb.instructions)
            out = []
            for inst in src:
                si = inst.sync_info
                waits = list(si.on_wait) if si is not None and si.on_wait else []
                if len(waits) > maxw:
                    keep = waits[-maxw:]
                    carry = waits[:-maxw]
                    for j in range(0, len(carry), maxw):
                        nop = nc.engines[inst.engine].nop(nofuse=True)
                        nc.cur_bb.bb.instructions.remove(nop.ins)
                        nop.ins.sync_info = mybir.SyncInfo(
                            on_wait=list(carry[j : j + maxw]), on_update=[]
                        )
                        out.append(nop.ins)
                    si.on_wait = keep
                    n_fixed += 1
                out.append(inst)
            bb.instructions[:] = out
    return n_fixed


# ---------------------------------------------------------------------------
# device program (identical on all 8 cores; per-core data differs)
# ---------------------------------------------------------------------------
def _build_nc(reps=1, stage=99):
    nc = bass.Bass("TRN2", target_bir_lowering=False, debug=False,
                   num_devices=NCORES)
    f32, bf16 = dt.float32, dt.bfloat16

    xT = nc.declare_dram_parameter("xT", [128, 4, S], bf16, isOutput=False)
    xqT = nc.declare_dram_parameter("xqT", [128, 4, QS], bf16, isOutput=False)
    wqkvh = nc.declare_dram_parameter("wqkvh", [128, 4, 768], bf16, isOutput=False)
    wph = nc.declare_dram_parameter("wph", [128, 2, D], bf16, isOutput=False)
    bqh = nc.declare_dram_parameter("bqh", [128, 2], f32, isOutput=False)
    bkh = nc.declare_dram_parameter("bkh", [128, 2], f32, isOutput=False)
    bvb = nc.declare_dram_parameter("bvb", [128, 256], f32, isOutput=False)
    mt = nc.declare_dram_parameter("mt", [128, NJT, QS], bf16, isOutput=False)
    ident = nc.declare_dram_parameter("ident", [128, 128], bf16, isOutput=False)
    out = nc.declare_dram_parameter("out", [QS, D], bf16, isOutput=True)

    with tile.TileContext(nc) as tc:
        with (
            tc.tile_pool(name="const", bufs=1) as constp,
            tc.tile_pool(name="kqp", bufs=2) as kqp,
            tc.tile_pool(name="mtp", bufs=2) as mtp,
            tc.tile_pool(name="vaugp", bufs=2) as vaugp,
            tc.tile_pool(name="chunk", bufs=3) as chp,
            tc.tile_pool(name="atm", bufs=3) as atmp,
            tc.tile_pool(name="norm", bufs=2) as normp,
            tc.tile_pool(name="ao", bufs=2) as aop,
            tc.tile_pool(name="psS", bufs=2, space="PSUM") as psS,
            tc.tile_pool(name="psV", bufs=1, space="PSUM") as psV,
            tc.tile_pool(name="psT", bufs=1, space="PSUM") as psTp,
            tc.tile_pool(name="psP", bufs=2, space="PSUM") as psP,
        ):
          pending_tail = []
          for rep in range(reps):
            # ---- resident loads ----
            wq_sb = constp.tile([128, 4, 768], bf16, tag="wqkv", name="wq_sb")
            nc.sync.dma_start(out=wq_sb[:], in_=wqkvh[:])
            bq_sb = constp.tile([128, 2], f32, tag="bq", name="bq_sb")
            nc.sync.dma_start(out=bq_sb[:], in_=bqh[:])
            bk_sb = constp.tile([128, 2], f32, tag="bk", name="bk_sb")
            nc.sync.dma_start(out=bk_sb[:], in_=bkh[:])
            bvb_sb = constp.tile([128, 256], f32, tag="bvb", name="bvb_sb")
            nc.sync.dma_start(out=bvb_sb[:], in_=bvb[:])
            id_sb = constp.tile([128, 128], bf16, tag="ident", name="id_sb")
            nc.sync.dma_start(out=id_sb[:], in_=ident[:])
            xt_sb = constp.tile([128, 4, S], bf16, tag="xt", name="xt_sb")
            for jb in range(4):
                nc.sync.dma_start(out=xt_sb[:, :, jb * 512 : (jb + 1) * 512],
                                  in_=xT[:, :, jb * 512 : (jb + 1) * 512])
            xqt_sb = constp.tile([128, 4, QS], bf16, tag="xqt", name="xqt_sb")
            nc.sync.dma_start(out=xqt_sb[:], in_=xqT[:])
            mt_sb = mtp.tile([128, NJT, QS], bf16, tag="mt")
            for piece in range(4):
                nc.sync.dma_start(
                    out=mt_sb[:, piece * 4 : (piece + 1) * 4, :],
                    in_=mt[:, piece * 4 : (piece + 1) * 4, :],
                )
            wp_sb = constp.tile([128, 2, D], bf16, tag="wp", name="wp_sb")
            nc.sync.dma_start(out=wp_sb[:], in_=wph[:])

            # persistent K^T / Q^T (2 head-pairs stacked on partitions), bf16
            kt2 = kqp.tile([128, 2, S], bf16, tag="kt2", name=f"kt2_{rep}")
            qt2 = kqp.tile([128, 2, QS], bf16, tag="qt2", name=f"qt2_{rep}")

            # ---- projection helpers (emitted interleaved below) ----
            def emit_kproj(kp, jb):
                kps = psP.tile([128, 512], f32, tag="pp",
                               name=f"kps_{rep}_{kp}_{jb}")
                for dtile in range(4):
                    nc.tensor.matmul(
                        kps[:],
                        wq_sb[:, dtile, 256 + kp * 128 : 256 + (kp + 1) * 128],
                        xt_sb[:, dtile, jb * 512 : (jb + 1) * 512],
                        start=(dtile == 0),
                        stop=(dtile == 3),
                    )
                nc.vector.tensor_scalar(
                    kt2[:, kp, jb * 512 : (jb + 1) * 512], kps[:],
                    bk_sb[:, kp : kp + 1], None, op0=ALU.add,
                )

            def emit_qproj(kp, qc):
                qps = psP.tile([128, 512], f32, tag="pp",
                               name=f"qps_{rep}_{kp}_{qc}")
                for dtile in range(4):
                    nc.tensor.matmul(
                        qps[:],
                        wq_sb[:, dtile, kp * 128 : (kp + 1) * 128],
                        xqt_sb[:, dtile, qc * 512 : (qc + 1) * 512],
                        start=(dtile == 0),
                        stop=(dtile == 3),
                    )
                nc.vector.tensor_scalar(
                    qt2[:, kp, qc * 512 : (qc + 1) * 512], qps[:],
                    bq_sb[:, kp : kp + 1], None, op0=ALU.add,
                )

            v_aug = vaugp.tile([128, NJT, HH * (HD + 1)], bf16, tag="vaug",
                               name=f"vaug_{rep}")
            nc.vector.memset(
                v_aug[:, :, :].rearrange("p t (h e) -> p t h e", e=HD + 1)[
                    :, :, :, HD : HD + 1
                ],
                1.0,
            )

            def emit_vproj(jt):
                vps = psP.tile([128, 256], f32, tag="pp",
                               name=f"vps_{rep}_{jt}")
                for dtile in range(4):
                    nc.tensor.matmul(
                        vps[:],
                        xt_sb[:, dtile, jt * 128 : (jt + 1) * 128],
                        wq_sb[:, dtile, 512:768],
                        start=(dtile == 0),
                        stop=(dtile == 3),
                    )
                dst = v_aug[:, jt, :].rearrange("p (h e) -> p h e", e=HD + 1)[
                    :, :, 0:HD
                ]
                nc.vector.tensor_add(
                    dst,
                    vps[:].rearrange("p (h e) -> p h e", e=HD),
                    bvb_sb[:].rearrange("p (h e) -> p h e", e=HD),
                )

            # ---- attention units, software-pipelined ----
            # unit u = (qh, hl). emit order: chunks(u) ... AV(u-1), norm(u-1)
            units = [(qh, hl) for qh in range(2) for hl in range(HH)]
            ao_tiles = {}
            unit_state = {}

            def emit_chunks(u, kt2=kt2, qt2=qt2, mt_sb=mt_sb,
                            unit_state=unit_state, rep=rep):
                qh, hl = units[u]
                kp, hp = hl // 2, hl % 2
                ktv = kt2[64 * hp : 64 * hp + 64, kp, :]
                qtv = qt2[64 * hp : 64 * hp + 64, kp,
                          qh * 512 : (qh + 1) * 512]
                atm = atmp.tile([128, NJT, 512], bf16, tag="atm",
                                name=f"atm_{rep}_{u}")
                for ch in range(NCHUNK):
                    sps = psS.tile([128, 2, 512], f32, tag="sc",
                                   name=f"sps_{rep}_{u}_{ch}")
                    at = chp.tile([128, 2, 512], bf16, tag="at",
                                  name=f"at_{rep}_{u}_{ch}")
                    for jc in range(2):
                        jt = 2 * ch + jc
                        nc.tensor.matmul(
                            sps[:, jc, :],
                            ktv[:, jt * 128 : (jt + 1) * 128],
                            qtv[:],
                            start=True, stop=True,
                        )
                    # prologue interleave: V/K/Q projections ride between the
                    # early units' score chunks. AV runs at pipeline depth 2
                    # for the first units, so v_aug is needed only at AV(0)
                    # (emitted after chunks(2)); K1-jb0/Q1-qc0 before unit 2's
                    # first scores, later K1 j-blocks two chunks ahead of use.
                    if u == 0 and stage >= 2:
                        emit_vproj(ch)
                    if u == 1:
                        if stage >= 2:
                            emit_vproj(8 + ch)
                        if stage >= 1 and ch == 3:
                            emit_kproj(1, 0)
                        if stage >= 1 and ch == 5:
                            emit_qproj(1, 0)
                    if u == 2 and stage >= 1:
                        if ch in (0, 2, 4):
                            emit_kproj(1, 1 + ch // 2)
                        elif ch == 6:
                            emit_qproj(0, 1)
                    if u == 3 and ch == 0 and stage >= 1:
                        emit_qproj(1, 1)
                    nc.scalar.activation(at[:], sps[:], AF.Exp, scale=SCALE)
                    muleng = nc.gpsimd if ch in (2, 5) else nc.vector
                    muleng.tensor_mul(
                        atm[:, 2 * ch : 2 * ch + 2, :], at[:],
                        mt_sb[:, 2 * ch : 2 * ch + 2,
                              qh * 512 : (qh + 1) * 512],
                    )
                unit_state[u] = atm

            def emit_av_norm(u, unit_state=unit_state,
                             ao_tiles=ao_tiles, v_aug=v_aug, rep=rep):
                qh, hl = units[u]
                atm = unit_state[u]
                avps = psV.tile([128, 4, HD + 1], f32, tag="av",
                                name=f"avps_{rep}_{u}")
                for qt in range(4):
                    for jt in range(NJT):
                        nc.tensor.matmul(
                            avps[:, qt, :],
                            atm[:, jt, qt * 128 : (qt + 1) * 128],
                            v_aug[:, jt, hl * (HD + 1) : (hl + 1) * (HD + 1)],
                            start=(jt == 0), stop=(jt == NJT - 1),
                        )
                if stage < 4:
                    return
                ao_sb = ao_tiles[qh]
                rec = normp.tile([128, 4], f32, tag="rec",
                                 name=f"rec_{rep}_{u}")
                nc.vector.reciprocal(
                    rec[:], avps[:, :, HD : HD + 1].rearrange("p q one -> p (q one)"))
                for qt in range(4):
                    nc.vector.tensor_scalar(
                        ao_sb[:, qt, hl * 64 : (hl + 1) * 64],
                        avps[:, qt, 0:HD], rec[:, qt : qt + 1], None, op0=ALU.mult,
                    )

            def emit_epilogue(qh, ao_tiles=ao_tiles, id_sb=id_sb,
                              wp_sb=wp_sb, rep=rep):
                # transpose ao + partial output projection for one seq-half
                if stage < 5:
                    return
                ao_sb = ao_tiles[qh]
                psT = psTp.tile([128, 4, 256], bf16, tag="tr",
                                name=f"psT_{rep}_{qh}")
                for st in range(4):
                    for ddt in range(2):
                        nc.tensor.transpose(
                            psT[:, st, ddt * 128 : (ddt + 1) * 128],
                            ao_sb[:, st, ddt * 128 : (ddt + 1) * 128],
                            id_sb[:],
                        )
                aot = normp.tile([128, 4, 256], bf16, tag="aoT",
                                 name=f"aoT_{rep}_{qh}")
                nc.vector.tensor_copy(aot[:], psT[:])
                for st in range(4):
                    ops = psP.tile([128, 512], f32, tag="pp",
                                   name=f"ops_{rep}_{qh}_{st}")
                    for ddt in range(2):
                        nc.tensor.matmul(
                            ops[:], aot[:, st, ddt * 128 : (ddt + 1) * 128],
                            wp_sb[:, ddt, :],
                            start=(ddt == 0), stop=(ddt == 1),
                        )
                    osb = normp.tile([128, D], bf16, tag="osb",
                                     name=f"osb_{rep}_{qh}_{st}")
                    nc.vector.tensor_copy(osb[:], ops[:])
                    nc.gpsimd.dma_start(
                        out=out[qh * 512 + st * 128 : qh * 512 + (st + 1) * 128, :],
                        in_=osb[:],
                    )

            if stage >= 1:
                for jb in range(4):
                    emit_kproj(0, jb)
                emit_qproj(0, 0)
            if stage >= 3:
                ao_tiles[0] = aop.tile([128, 4, 256], bf16, tag="ao",
                                       name=f"ao0_{rep}")
                ao_tiles[1] = aop.tile([128, 4, 256], bf16, tag="ao",
                                       name=f"ao1_{rep}")
                # deferred tail of the previous rep: its last unit's chunks
                # ride in this rep's head windows (this rep's K0/Q0 were just
                # emitted and execute in the previous unit-6 window's slack)
                if pending_tail:
                    t_ch7, t_av7, t_ep1 = pending_tail.pop()
                else:
                    t_ch7 = t_av7 = t_ep1 = None
                if t_ch7 is not None:
                    t_ch7()
                emit_chunks(0)
                if t_av7 is not None:
                    t_av7()
                emit_chunks(1)
                emit_chunks(2)
                emit_av_norm(0)
                emit_chunks(3)
                emit_av_norm(1)
                if t_ep1 is not None:
                    t_ep1()
                emit_chunks(4)
                emit_av_norm(2)
                emit_av_norm(3)
                emit_chunks(5)
                emit_av_norm(4)
                emit_epilogue(0)
                emit_chunks(6)
                emit_av_norm(5)

                def _t_ch7(c=emit_chunks, a=emit_av_norm):
                    c(7)
                    a(6)

                def _t_av7(a=emit_av_norm):
                    a(7)

                def _t_ep1(e=emit_epilogue):
                    e(1)

                pending_tail.append((_t_ch7, _t_av7, _t_ep1))
          if pending_tail:
              for f in pending_tail.pop():
                  f()

    _split_sync_waits(nc)
    return nc


_NC_CACHE = {}


def _get_nc(reps=1, stage=99):
    if (reps, stage) not in _NC_CACHE:
        _NC_CACHE[(reps, stage)] = _build_nc(reps, stage)
    return _NC_CACHE[(reps, stage)]


# ---------------------------------------------------------------------------
# host wrapper
# ---------------------------------------------------------------------------
def _prep_inputs(x, routes, w_qkv, b_qkv, w_proj, b_proj):
    x = np.asarray(x, dtype=np.float32)
    routes = np.asarray(routes)
    w_qkv = np.asarray(w_qkv, dtype=np.float32)
    b_qkv = np.asarray(b_qkv, dtype=np.float32)
    w_proj = np.asarray(w_proj, dtype=np.float32)
    b_proj = np.asarray(b_proj, dtype=np.float32)
    bf = ml_dtypes.bfloat16

    r = np.clip(routes[:S].astype(np.int64), 0, S - 1)
    # multiplicity mask M[s, j] = count of j in routes[s]
    flat = (np.arange(S, dtype=np.int64)[:, None] * S + r).ravel()
    M = np.bincount(flat, minlength=S * S).reshape(S, S).astype(np.float32)

    def t_layout(w, n_out, ntile):  # w: (n_out, 512) -> (128, ntile, n_out)
        return np.ascontiguousarray(
            w.T.reshape(ntile, 128, n_out).transpose(1, 0, 2).astype(bf)
        )

    id128 = np.eye(128, dtype=np.float32).astype(bf)

    in_maps = []
    for c in range(NCORES):
        b, hh, sh = c // 4, (c % 4) // 2, c % 2
        hd0 = hh * 256
        # weights for this head-half: Q rows [hd0, hd0+256), K rows
        # [D+hd0, ...), V rows [2D+hd0, ...) -> [128, 4, 768] (q|k|v)
        wsel = np.concatenate(
            [w_qkv[hd0 : hd0 + 256],
             w_qkv[D + hd0 : D + hd0 + 256],
             w_qkv[2 * D + hd0 : 2 * D + hd0 + 256]], axis=0
        )  # (768, 512)
        wqkvh = t_layout(wsel, 768, 4)
        # wproj columns for this head-half's ao dims -> [128, 2, 512]
        wph = np.ascontiguousarray(
            w_proj[:, hd0 : hd0 + 256].T.reshape(2, 128, D)
            .transpose(1, 0, 2).astype(bf)
        )
        bqh = np.ascontiguousarray(
            b_qkv[hd0 : hd0 + 256].reshape(2, 128).T.astype(np.float32))
        bkh = np.ascontiguousarray(
            b_qkv[D + hd0 : D + hd0 + 256].reshape(2, 128).T.astype(np.float32))
        bvbh = np.ascontiguousarray(
            np.tile(b_qkv[2 * D + hd0 : 2 * D + hd0 + 256], (128, 1))
        ).astype(np.float32)

        xb = x[b]  # (S, D)
        xTc = np.ascontiguousarray(
            xb.T.reshape(4, 128, S).transpose(1, 0, 2).astype(bf))
        s0 = sh * QS
        xqTc = np.ascontiguousarray(
            xb[s0 : s0 + QS].T.reshape(4, 128, QS).transpose(1, 0, 2).astype(bf)
        )
        # mt[p, t, q] = M[s0+q, t*128+p]
        mtc = M[s0 : s0 + QS].T.reshape(NJT, 128, QS).transpose(1, 0, 2)
        mtc = np.ascontiguousarray(mtc.astype(bf))
        in_maps.append(
            {
                "xT": xTc, "xqT": xqTc, "wqkvh": wqkvh, "wph": wph,
                "bqh": bqh, "bkh": bkh, "bvb": bvbh, "mt": mtc,
                "ident": id128,
            }
        )
    return in_maps


def run_cores(in_maps, reps=1, stage=99, **kwargs):
    nc = _get_nc(reps, stage)
    return run_bass_kernel_spmd(nc, in_maps, list(range(NCORES)), **kwargs)


def kernel(x, routes, w_qkv, b_qkv, w_proj, b_proj):
    b_proj = np.asarray(b_proj, dtype=np.float32)
    in_maps = _prep_inputs(x, routes, w_qkv, b_qkv, w_proj, b_proj)
    res = run_cores(in_maps)
    out = np.empty((B, S, D), dtype=np.float32)
    for b in range(B):
        for sh in range(2):
            s0 = sh * QS
            out[b, s0 : s0 + QS] = (
                res.results[b * 4 + sh]["out"].astype(np.float32)
                + res.results[b * 4 + 2 + sh]["out"].astype(np.float32)
                + b_proj
            )
    return out

